# revision 2
# baseline (speedup 1.0000x reference)
"""TRN2 Bass kernel for nn_MetaBaseline (DN4-style local-descriptor kNN), v2.

Reference computation (per batch b):
  q = normalize(input1[b].reshape(75, 100, 640), axis=-1)      # query patches
  s = normalize(input2[b].reshape(2500, 640), axis=-1)         # support descs
  scores = q_patches @ s.T                                     # [7500, 2500]
  per way group g (columns [500g, 500g+500)): top-k per row, mean,
  then sum over the 100 patches of each query -> out [75, 5].

Sharding: data-parallel over (b, query-quarter): 8 cores, each handles one
batch's quarter of queries (19 queries padded) with that batch's full
support replicated.

v2 strategy (hardware-measured, see microbench):
  - all score matmul operands are fp8 (e4m3). q is quantized AND transposed
    on the host (pure layout prep); its norm is computed on-device from a
    row-major fp8 copy and folded into the final per-patch scale
    1/(16*k*|q^|) (top-k per row is invariant to positive row scaling, so
    q is never normalized before the matmul).
  - s is normalized on-device in row layout on the scalar engine (scaled
    x16 to keep fp8 relative precision) into bf16, PE-transposed in bf16
    (1 cycle/row, 88ns/tile measured; fp8 transposes need 2-byte-strided
    PSUM), and quantized to fp8 by the packed 5-chunk PSUM eviction copy.
  - scores per (m-tile, way): chunks 0-3 as two DoubleRow K=256 pairs
    (x2 250-col output halves, ~107ns each measured) + chunk 4 as one
    normal fp8 N=500 matmul (~210ns): ~640ns vs 1125ns for f32r.
  - top-8 per (patch, way) via DVE max8 straight from the PSUM score bank
    (592ns measured, identical to SBUF reads, so no eviction); pass 4
    reduces top-k, scales by qinv, and accumulates per-query sums with a
    small f32 indicator matmul.
  - engine balance: ACT does squares/sqrt/muls/evictions (~49us), DVE does
    max8/reciprocal/top-k reduce (~49us), PE does scores+transposes (~58us).
    q-norm squares are spread over passes 0-3 to keep pass 0 from going
    ACT-bound.
"""
import os
from contextlib import ExitStack

import numpy as np

import concourse.bass as bass  # noqa: F401
import concourse.mybir as mybir
import concourse.tile as tile
from concourse import bacc
from concourse.bass_utils import run_bass_kernel_spmd

# Problem geometry (hardcoded per contest rules)
B, Q, WAY, SHOT, H, W, C = 2, 75, 5, 5, 10, 10, 640
HW = H * W               # 100 patches per query / support image
NQ = 19                  # queries per core (4 cores x 19 = 76 >= 75)
MT = 15                  # patch M-tiles of 128 -> 1920 rows (1900 real)
PAD_P = MT * 128
NS = WAY * SHOT * HW     # 2500 support descriptors per batch
ST = 20                  # support tiles of 128 -> 2560 rows
PAD_S = ST * 128
KC = 5                   # C chunks of 128 (640 = 5*128)
P = 128
NW = SHOT * HW           # 500 support descriptors per way group
NH = NW // 2             # DoubleRow moving half (250)
N_CORES = 8
N_WARM = int(os.environ.get("N_WARM", "20"))
DR_MODE = os.environ.get("DR_MODE", "dr6")  # "dr6" | "fp8"
NOQ = os.environ.get("NOQ", "0") == "1"     # bisect: skip q-norm path
NOSC = os.environ.get("NOSC", "0") == "1"   # bisect: skip scores/top-k
NOPREP = os.environ.get("NOPREP", "0") == "1"  # bisect: skip s-prep ops
SSCALE = 16.0            # fp8 headroom scale folded into s normalization

f32 = mybir.dt.float32
bf16 = mybir.dt.bfloat16
fp8 = mybir.dt.float8e4
np8 = mybir.dt.np(fp8)
DRM = mybir.MatmulPerfMode.DoubleRow
AF = mybir.ActivationFunctionType

_prog_cache: dict[int, object] = {}


def _build(k: int):
    """Build + compile the per-core SPMD program for neighbor_k == k."""
    assert 1 <= k <= 8, f"neighbor_k={k} not supported (need 1..8)"
    nc = bacc.Bacc("TRN2", target_bir_lowering=False, debug=False)

    qT_d = nc.dram_tensor("qT", [P, KC * PAD_P], fp8, kind="ExternalInput").ap()
    qr_d = nc.dram_tensor("qr", [PAD_P, C], fp8, kind="ExternalInput").ap()
    s_d = nc.dram_tensor("s", [PAD_S, C], fp8, kind="ExternalInput").ap()
    ind_d = nc.dram_tensor("ind", [P, MT * NQ], f32, kind="ExternalInput").ap()
    idb_d = nc.dram_tensor("idb", [P, P], bf16, kind="ExternalInput").ap()

    out_d = nc.dram_tensor("out", [NQ, WAY], f32, kind="ExternalOutput").ap()

    with tile.TileContext(nc) as tc:
        with ExitStack() as ctx:
            const = ctx.enter_context(tc.tile_pool(name="const", bufs=1))
            big = ctx.enter_context(tc.tile_pool(name="big", bufs=1))
            loads = ctx.enter_context(tc.tile_pool(name="loads", bufs=6))
            small = ctx.enter_context(tc.tile_pool(name="small", bufs=4))
            mxp = ctx.enter_context(tc.tile_pool(name="mxp", bufs=MT))
            outp = ctx.enter_context(
                tc.tile_pool(name="outp", bufs=1, space="PSUM"))
            tpp = ctx.enter_context(
                tc.tile_pool(name="tpp", bufs=2, space="PSUM"))
            spp = ctx.enter_context(
                tc.tile_pool(name="spp", bufs=4, space="PSUM"))

            idbf = const.tile([P, P], bf16)
            ind_sb = const.tile([P, MT * NQ], f32)
            qsum = const.tile([P, MT], f32)
            qinv = const.tile([P, MT], f32)
            ssums = const.tile([P, ST], f32)
            sinvs = const.tile([P, ST], f32)

            # fp8 chunk-banded transposed operands: chunk c of each tensor
            # occupies its own column band so a DoubleRow pair is a strided
            # [P, 2, n] AP and a packed PSUM transpose evicts in one copy
            s_T = big.tile([P, 6 * PAD_S], fp8, name="s_T")
            q_T = big.tile([P, 6 * PAD_P], fp8, name="q_T")
            s3 = s_T.rearrange("p (c n) -> p c n", c=6)
            q3 = q_T.rearrange("p (c n) -> p c n", c=6)
            # row-major fp8 copies for the norm chains
            q_rows = big.tile([P, MT * C], fp8, name="q_rows")
            s_rows = big.tile([P, ST * C], fp8, name="s_rows")

            out_ps = outp.tile([NQ, WAY], f32)

            # ---- warmups: ACT tables + PE pipeline ----
            # idb is the first DMA so the PE starts ramping on identity
            # transposes almost immediately (a non-permutation rhs in 2-byte
            # transpose mode is illegal and wedges the PE).
            nc.sync.dma_start(out=idbf, in_=idb_d)
            wf = const.tile([P, 1], f32, name="wf")
            nc.vector.memset(wf, 1.0)
            wsq = small.tile([P, 1], f32, tag="snrm")
            nc.scalar.activation(wsq, wf, AF.Square)
            nc.scalar.sqrt(wsq, wf)
            wps = tpp.tile([P, KC * P], bf16, tag="tp")
            for i in range(N_WARM):
                nc.tensor.matmul(
                    wps[:, (i % KC) * P:(i % KC + 1) * P], idbf, idbf,
                    is_transpose=True)

            def s_dma_group(g, eng=None):
                # 4 tiles per DMA: dst [128, 4, 640] <- src [4, 128, 640]
                (eng or nc.sync).dma_start(
                    out=s_rows[:, 4 * g * C:4 * (g + 1) * C].rearrange(
                        "p (t c) -> p t c", t=4),
                    in_=s_d[4 * g * P:4 * (g + 1) * P, :].rearrange(
                        "(t p) c -> p t c", t=4))

            def q_dma_group(g, eng=None):
                # 5 tiles per DMA
                (eng or nc.sync).dma_start(
                    out=q_rows[:, 5 * g * C:5 * (g + 1) * C].rearrange(
                        "p (t c) -> p t c", t=5),
                    in_=qr_d[5 * g * P:5 * (g + 1) * P, :].rearrange(
                        "(t p) c -> p t c", t=5))

            def s_sq(t):
                # row sum-of-squares, accumulated into column t of ssums
                sq = loads.tile([P, C], f32, tag="sq", name=f"ssq{t}")
                nc.scalar.activation(sq, s_rows[:, t * C:(t + 1) * C],
                                     AF.Square, accum_out=ssums[:, t:t + 1])

            def s_norm2(t0):
                # batched sqrt+reciprocal for tiles t0, t0+1:
                # sinvs = SSCALE / |s^|
                sl = slice(t0, t0 + 2)
                snrm = small.tile([P, 2], f32, tag="snrm")
                nc.scalar.activation(snrm, ssums[:, sl], AF.Sqrt,
                                     scale=1.0 / (SSCALE * SSCALE))
                nc.vector.reciprocal(sinvs[:, sl], snrm)

            def s_scale(t, evict_on_dve=False, scale_on_dve=False):
                x = s_rows[:, t * C:(t + 1) * C]
                s_n = loads.tile([P, C], bf16, tag="s_n", name=f"sn{t}")
                if scale_on_dve:
                    nc.vector.tensor_scalar_mul(s_n, x, sinvs[:, t:t + 1])
                else:
                    nc.scalar.mul(s_n, x, sinvs[:, t:t + 1])
                ps = tpp.tile([P, KC * P], bf16, tag="tp", name=f"ps{t}")
                for c in range(KC):
                    nc.tensor.matmul(
                        ps[:, c * P:(c + 1) * P], s_n[:, c * P:(c + 1) * P],
                        idbf, is_transpose=True)
                dst = s3[:, 0:KC, t * P:(t + 1) * P]
                src = ps.rearrange("p (c n) -> p c n", c=KC)
                if evict_on_dve:
                    nc.vector.tensor_copy(dst, src)
                else:
                    nc.scalar.copy(dst, src)

            def q_norm(m):
                if NOQ:
                    return
                sq = loads.tile([P, C], f32, tag="sq", name=f"qsq{m}")
                nc.scalar.activation(sq, q_rows[:, m * C:(m + 1) * C],
                                     AF.Square, accum_out=qsum[:, m:m + 1])

            def s_dma2(t0):
                # 2 tiles per DMA
                nc.sync.dma_start(
                    out=s_rows[:, t0 * C:(t0 + 2) * C].rearrange(
                        "p (t c) -> p t c", t=2),
                    in_=s_d[t0 * P:(t0 + 2) * P, :].rearrange(
                        "(t p) c -> p t c", t=2))

            def qT_dma(c0, c1):
                # one DMA: cols c0..c1 of all 5 real chunk bands
                nc.sync.dma_start(
                    out=q3[:, 0:KC, c0:c1],
                    in_=qT_d.rearrange("p (c n) -> p c n", c=KC)[:, :, c0:c1])

            # ---- prologue ----
            # Everything rides the single ~140GB/s Sync HW queue, so order
            # strictly by first use: s tiles 0-1, 2-3, qT cols 0-640 (covers
            # m-tiles 0-4), s tiles 4-5, qT cols 640-1280, s 6-7, qT rest.
            # The zero DoubleRow bands are DVE memsets, split so the strict
            # FIFO DVE queue isn't blocked ahead of the norm reciprocals.
            nc.vector.memset(s3[:, 5, 0:640], 0.0)
            nc.vector.memset(q3[:, 5, 0:640], 0.0)
            s_dma2(0)
            s_dma2(2)
            qT_dma(0, 640)
            s_dma2(4)
            qT_dma(640, 1280)
            s_dma2(6)
            qT_dma(1280, PAD_P)
            # norm chains for tiles 0-3; scales 1,3 on DVE so the serial
            # scalar-engine chain isn't the prologue critical path
            s_sq(0)
            s_sq(1)
            s_norm2(0)
            s_scale(0)
            s_scale(1, scale_on_dve=True, evict_on_dve=True)
            s_sq(2)
            s_sq(3)
            s_norm2(2)
            s_scale(2)
            s_scale(3, scale_on_dve=True, evict_on_dve=True)
            nc.vector.memset(s3[:, 5, 640:PAD_S], 0.0)
            nc.vector.memset(q3[:, 5, 640:PAD_P], 0.0)

            def scores(psc, m, w):
                if DR_MODE == "dr6":
                    # per-half accumulation groups; 3 uniform DoubleRow pairs
                    # (chunk pair (4,5) has a zero q band), the HW-verified
                    # microbench phase-D structure
                    for nh in range(2):
                        o = psc[:, nh * NH:(nh + 1) * NH]
                        for cp in range(3):
                            nc.tensor.matmul(
                                o,
                                q3[:, 2 * cp:2 * cp + 2, m * P:(m + 1) * P],
                                s3[:, 2 * cp:2 * cp + 2,
                                   w * NW + nh * NH:w * NW + (nh + 1) * NH],
                                start=(cp == 0), stop=(cp == 2), perf_mode=DRM)
                else:
                    for c in range(KC):
                        nc.tensor.matmul(
                            psc, q3[:, c, m * P:(m + 1) * P],
                            s3[:, c, w * NW:(w + 1) * NW],
                            start=(c == 0), stop=(c == KC - 1))

            # pass-w<4 schedule (prepping s-tiles T..T+3 for way w+1):
            # pass 0 runs late (prologue DMAs still landing): sqs at
            # m=6,8,10,11, norms paired, scales at 9,11,12,13.  Later passes
            # run early.  q rows stream in at pass starts; q-norm squares run
            # 5 per pass on passes 1-3.
            mxs = [None] * MT
            prev = [None, None]
            for w in range(WAY):
                T = 4 * (w + 1)
                for m in range(MT):
                    if not NOQ and m == 0 and w in (0, 1, 2):
                        q_dma_group(w)
                    if w == 1 and m == 3:
                        nc.sync.dma_start(out=ind_sb, in_=ind_d)
                    if w == 0:
                        if m == 7 and not NOPREP:
                            s_dma_group(2)
                        if m == 6:
                            s_sq(T)
                        elif m == 8:
                            s_sq(T + 1)
                            s_norm2(T)
                        elif m == 9:
                            s_scale(T)
                        elif m == 10:
                            s_sq(T + 2)
                        elif m == 11:
                            s_sq(T + 3)
                            s_norm2(T + 2)
                            s_scale(T + 1, scale_on_dve=True,
                                    evict_on_dve=True)
                        elif m == 12:
                            s_scale(T + 2)
                        elif m == 13:
                            s_scale(T + 3, evict_on_dve=True)
                    elif w < 4:
                        if m == 7 and w < 3:
                            s_dma_group(w + 2)
                        if m in (1, 2, 3, 4):
                            s_sq(T + m - 1)
                            if m == 2:
                                s_norm2(T)
                            elif m == 4:
                                s_norm2(T + 2)
                        elif m in (5, 7, 9, 11):
                            s_scale(T + (m - 5) // 2,
                                    evict_on_dve=(m in (7, 11)))
                        if m in (6, 8, 10, 12, 13):
                            j = 5 * (w - 1) + {6: 0, 8: 1, 10: 2,
                                               12: 3, 13: 4}[m]
                            if j < MT:
                                q_norm(j)
                    if w == 3 and m == 13:
                        if NOQ:
                            nc.vector.memset(qinv, 0.01)
                        else:
                            # qinv = 1/(SSCALE*k*|q^|)
                            kn = small.tile([P, MT], f32, tag="kn")
                            nc.scalar.activation(
                                kn, qsum, AF.Sqrt,
                                scale=float(SSCALE * SSCALE * k * k))
                            nc.vector.reciprocal(qinv, kn)
                    if w == 0:
                        mxs[m] = mxp.tile([P, WAY * 8], f32, tag="mx",
                                          name=f"mx{m}")
                    if NOSC:
                        continue
                    psc = spp.tile([P, NW], f32, tag="psc",
                                   name=f"psc{m}_{w}")
                    scores(psc, m, w)
                    nc.vector.max(mxs[m][:, w * 8:(w + 1) * 8], psc)
                    if w == WAY - 1:
                        tsum = small.tile([P, WAY], f32, tag="tsum")
                        nc.vector.tensor_reduce(
                            tsum,
                            mxs[m].rearrange("p (w j) -> p w j", w=WAY)[:, :, :k],
                            axis=mybir.AxisListType.X,
                            op=mybir.AluOpType.add,
                        )
                        scaled = small.tile([P, WAY], f32, tag="scaled")
                        nc.scalar.mul(scaled, tsum, qinv[:, m:m + 1])
                        if prev[0] is not None:
                            nc.tensor.matmul(
                                out_ps,
                                ind_sb[:, prev[1] * NQ:(prev[1] + 1) * NQ],
                                prev[0], start=(prev[1] == 0), stop=False)
                        prev = [scaled, m]
            out_sb = small.tile([NQ, WAY], f32, tag="out_sb")
            if NOSC:
                nc.vector.memset(out_sb, 0.0)
            else:
                nc.tensor.matmul(
                    out_ps, ind_sb[:, prev[1] * NQ:(prev[1] + 1) * NQ],
                    prev[0], start=False, stop=True)
                nc.scalar.copy(out_sb, out_ps)
            nc.sync.dma_start(out=out_d, in_=out_sb)

    nc.compile()
    return nc


def get_program(k: int):
    if k not in _prog_cache:
        _prog_cache[k] = _build(k)
    return _prog_cache[k]


def make_in_maps(input1: np.ndarray, input2: np.ndarray):
    """Shard full inputs into per-core input maps (pure layout prep)."""
    input1 = np.ascontiguousarray(np.asarray(input1), dtype=np.float32)
    input2 = np.ascontiguousarray(np.asarray(input2), dtype=np.float32)
    idb = np.eye(P, dtype=mybir.dt.np(bf16))
    in_maps = []
    for core in range(N_CORES):
        b = core // 4
        qs = (core % 4) * NQ
        qe = min(Q, qs + NQ)
        nq = qe - qs
        qdat = input1[b].reshape(Q, HW, C)[qs:qe].reshape(-1, C)
        qfull = np.ones((PAD_P, C), np.float32)
        qfull[: nq * HW] = qdat
        q8 = qfull.astype(np8)
        # chunk-banded transpose: qT[p, c*PAD_P + j] = q8[j, c*128 + p]
        # (band 5, the DoubleRow partner of the zero s band, is device-zeroed)
        qT = np.ascontiguousarray(
            q8.reshape(PAD_P, KC, P).transpose(2, 1, 0)).reshape(P, KC * PAD_P)
        sfull = np.ones((PAD_S, C), np.float32)
        sfull[:NS] = input2[b].reshape(NS, C)
        s8 = sfull.astype(np8)
        # indicator: patch row p of M-tile t belongs to query (t*128+p)//HW
        ind = np.zeros((P, MT * NQ), np.float32)
        g = np.arange(MT * P)
        j = g // HW
        valid = j < nq
        ind[g[valid] % P, (g[valid] // P) * NQ + j[valid]] = 1.0
        in_maps.append({"qT": qT, "qr": q8, "s": s8, "ind": ind, "idb": idb})
    return in_maps


def gather_out(results) -> np.ndarray:
    out = np.zeros((B, Q, WAY), np.float32)
    for core in range(N_CORES):
        b = core // 4
        qs = (core % 4) * NQ
        n = min(Q, qs + NQ) - qs
        out[b, qs:qs + n] = results[core]["out"][:n]
    return out


def kernel(input1, input2, neighbor_k):
    k = int(np.asarray(neighbor_k))
    nc = get_program(k)
    in_maps = make_in_maps(input1, input2)
    # the axon-tunneled device occasionally reports a transient
    # "unrecoverable" state right after a previous process's teardown;
    # it recovers within seconds, so retry a couple of times
    import time
    last = None
    for attempt in range(3):
        try:
            res = run_bass_kernel_spmd(
                nc, in_maps, core_ids=list(range(N_CORES)))
            return gather_out(res.results)
        except Exception as e:  # noqa: BLE001
            last = e
            if attempt < 2:
                time.sleep(20.0 * (attempt + 1))
    raise last
